# revision 46
# baseline (speedup 1.0000x reference)
"""Trainium2 Bass kernel for nn_AttentionBlock (adaLN-modulated GroupNorm attention).

Sharding: data-parallel over batch B=8 -> one batch per NeuronCore (8 cores).
Each core runs the full block for its batch:
  groupnorm(32 groups) -> adaLN modulate -> qkv matmul -> 8-head attention
  (softmax over keys) -> proj matmul -> gated residual.

Final design (v6): restructured for PE density / HAM clock-gate warmth:
  - q, k computed as [cout, T] tiles (type-major permuted channel order) so
    head h's q/k live at partition offset 64*(h%2) of tile h//2.
  - v computed TRANSPOSED directly by the qkv matmul (lhsT = xm chunk,
    rhs = v-weight columns) -> vt tiles [128 s, 8*65]: per head 65 cols =
    [v channels | ones], so PV's U output carries the softmax denominator
    in partition row 64 (no ones appended via copies, no PE transposes).
  - fp8 (e4m3) DoubleRow matmuls for qkv, vT and proj (weights pre-scaled
    x64 to clear the fp8 normal floor; rescaled for free in the evictions);
    scores and PV stay bf16. Halves those matmul cycles AND the weight DMA.
  - flat cross-pair software pipeline: PV runs 4 slots behind scores, all
    pair-boundary work (stage copy / broadcast / reciprocal / normalize
    muls) is spread across fixed slot offsets so the PE never bunches.
  - exp split: head A of each pair on ScalarE (exact exp), head B on DVE
    via the Schraudolph bit trick (bf16 bits ~= int16(A*s + B)); the
    uniform ~0.3% scale error cancels in softmax normalization.
  - normalize: ONE ACT copy of U[0:65] to an SBUF stage tile (releases the
    U PSUM banks early) -> PE rank-1 e64 broadcast of the denom row -> DVE
    fast reciprocal -> multiply stage*recip into the fp8 a-tiles (head A on
    gpsimd, head B on DVE using its partition-shifted output).
  - proj bias folded into the proj matmul (rank-1: lhsT=pb, rhs=ones);
    gated residual fused in one DVE scalar_tensor_tensor per chunk.
  - PE warmup burst of dummy matmuls before qkv to lift the HAM clock gate.

Attention matmuls run in bf16, qkv/vT/proj in fp8 DoubleRow (fp32 PSUM
accumulation everywhere); groupnorm statistics and the residual stay fp32.
"""

import numpy as np

import concourse.bass as bass
import concourse.tile as tile
from concourse import bacc, mybir
from concourse.bass_utils import run_bass_kernel_spmd

AF = mybir.ActivationFunctionType
ALU = mybir.AluOpType
f32 = mybir.dt.float32
bf16 = mybir.dt.bfloat16
i16 = mybir.dt.int16
f8 = mybir.dt.float8e4
WSCALE = 64.0        # fp8 weights pre-scaled by 64 (dodges fp8 normal floor)

B, C, HH, WW, E = 8, 512, 32, 32, 512
HEADS, G = 8, 32
T = HH * WW          # 1024
CH = C // HEADS      # 64
NC_ = C // 128       # 4 channel chunks
NT = T // 512        # 2 t-chunks of 512
NS = T // 128        # 8 s-chunks of 128
EPS = 1e-5
WARMUP_PRE = 12      # PE warmup matmuls at t=0 (HAM clock-gate lift)
WARMUP_A = 40        # warmup covering the x-DMA/stats window
WARMUP_B = 24        # warmup bridging xm latency into qkv
EXP_SPLIT = True     # head B exp on DVE (Schraudolph); False -> all on ACT
# Schraudolph: bf16 bits of exp(0.125*s) ~= int16(EXP_A*s + EXP_B)
EXP_A = 128.0 * 0.125 / float(np.log(2.0))
EXP_B = 16256.0 - 128.0 * 0.043


def _perm():
    """new[512*ty + 64*h + r] = orig[192*h + 64*ty + r] (head-major -> type-major)."""
    p = np.empty(3 * C, np.int64)
    for h in range(HEADS):
        for ty in range(3):
            p[512 * ty + 64 * h : 512 * ty + 64 * h + 64] = (
                192 * h + 64 * ty + np.arange(64)
            )
    return p


def _build_program():
    nc = bacc.Bacc("TRN2", target_bir_lowering=False, debug=False, num_devices=8)

    # ---- DRAM parameters (per-core shards; weights replicated, bf16) ----
    x_d = nc.declare_dram_parameter("x", [C, T], f32, isOutput=False)
    emb_d = nc.declare_dram_parameter("emb", [E], f32, isOutput=False)
    qw_d = nc.declare_dram_parameter("qkv_wT", [C, 3 * C], f8, isOutput=False)
    qb_d = nc.declare_dram_parameter("qkv_b", [3 * C], f32, isOutput=False)
    vb_d = nc.declare_dram_parameter("vbrow", [1, C], bf16, isOutput=False)
    aw_d = nc.declare_dram_parameter("ada_wT", [E, 3 * C], f8, isOutput=False)
    ab_d = nc.declare_dram_parameter("ada_b", [3 * C], f32, isOutput=False)
    pw_d = nc.declare_dram_parameter("proj_wT", [C, C], f8, isOutput=False)
    pb_d = nc.declare_dram_parameter("pbrow", [1, C], f8, isOutput=False)
    gind_d = nc.declare_dram_parameter("gind", [128, 8], f32, isOutput=False)
    gindT_d = nc.declare_dram_parameter("gindT", [8, 128], f32, isOutput=False)
    out_d = nc.declare_dram_parameter("out", [C, T], f32, isOutput=True)

    from contextlib import ExitStack

    with tile.TileContext(nc) as tc, ExitStack() as ctx:
        ctx.enter_context(
            nc.allow_low_precision(reason="bf16 matmul inputs; fp32 accumulate")
        )
        P = ctx.enter_context(tc.tile_pool(name="persist", bufs=1))
        # PSUM: "sc" slots [128,1024] f32 = 2 banks x 2 bufs; "u" = 2 banks x 2
        PSM = ctx.enter_context(tc.tile_pool(name="psm", bufs=2, space="PSUM"))
        PSU = ctx.enter_context(tc.tile_pool(name="psu", bufs=2, space="PSUM"))
        EXPP = ctx.enter_context(tc.tile_pool(name="expp", bufs=6))
        EXPI = ctx.enter_context(tc.tile_pool(name="expi", bufs=6))
        ANP = ctx.enter_context(tc.tile_pool(name="anp", bufs=2))

        # ---- persistent SBUF tiles + input DMAs (ordered by need time) ----
        gind_sb = P.tile([128, 8], f32, tag="gind")
        gindT_sb = P.tile([8, 128], f32, tag="gindT")
        emb_sb = P.tile([128, 4], f32, tag="emb")
        qb_sb = P.tile([128, 12], f32, tag="qb")
        ab_sb = P.tile([128, 12], f32, tag="ab")
        vb_sb = P.tile([1, C], bf16, tag="vb")
        pbr_sb = P.tile([1, C], f8, tag="pbr")

        nc.sync.dma_start(out=gind_sb, in_=gind_d.ap())
        nc.sync.dma_start(out=gindT_sb, in_=gindT_d.ap())
        nc.sync.dma_start(out=emb_sb, in_=emb_d.ap().rearrange("(f p) -> p f", p=128))
        nc.sync.dma_start(out=qb_sb, in_=qb_d.ap().rearrange("(f p) -> p f", p=128))
        nc.sync.dma_start(out=ab_sb, in_=ab_d.ap().rearrange("(f p) -> p f", p=128))
        nc.sync.dma_start(out=vb_sb, in_=vb_d.ap())
        nc.sync.dma_start(out=pbr_sb, in_=pb_d.ap())

        xf = []
        for i in range(NC_):
            t_ = P.tile([128, T], f32, tag=f"xf{i}", name=f"xf{i}")
            for hh in range(2):
                nc.sync.dma_start(
                    out=t_[:, 512 * hh : 512 * (hh + 1)],
                    in_=x_d.ap()[128 * i : 128 * (i + 1), 512 * hh : 512 * (hh + 1)],
                )
            xf.append(t_)
        aw = []
        for j in range(4):
            t_ = P.tile([128, 3 * C], f8, tag=f"aw{j}", name=f"aw{j}")
            nc.sync.dma_start(out=t_, in_=aw_d.ap()[128 * j : 128 * (j + 1), :])
            aw.append(t_)
        # qw/pw as DoubleRow pair tiles: [:, :1536] = chunk 2jj, [:, 1536:] = 2jj+1
        qw8p = []
        for jj in range(2):
            t_ = P.tile([128, 2 * 3 * C], f8, tag=f"qw8p{jj}", name=f"qw8p{jj}")
            for i2 in range(2):
                j = 2 * jj + i2
                nc.sync.dma_start(
                    out=t_[:, 1536 * i2 : 1536 * (i2 + 1)],
                    in_=qw_d.ap()[128 * j : 128 * (j + 1), :],
                )
            qw8p.append(t_)
        pw8p = []
        for jj in range(2):
            t_ = P.tile([128, 2 * C], f8, tag=f"pw8p{jj}", name=f"pw8p{jj}")
            for i2 in range(2):
                j = 2 * jj + i2
                nc.sync.dma_start(
                    out=t_[:, 512 * i2 : 512 * (i2 + 1)],
                    in_=pw_d.ap()[128 * j : 128 * (j + 1), :],
                )
            pw8p.append(t_)

        # small constants built on-chip
        ones_row = P.tile([1, T], f8, tag="ones_row")
        nc.vector.memset(ones_row, 1.0)
        dummy_sb = P.tile([128, 512], bf16, tag="dummy")
        nc.vector.memset(dummy_sb, 0.0)
        # e64: [65, 64] with row 64 = ones -> PE broadcast of denominator row
        e64 = P.tile([65, 64], bf16, tag="e64")
        nc.vector.memset(e64, 0.0)
        nc.vector.memset(e64[64:65, :], 1.0)
        # rca: per-head-slot staging for the denominator row (row 64); rows
        # 0:64 stay zero so the e64 broadcast matmul sees no garbage
        rca = {}
        for off in (0, 64):
            t_ = P.tile([65, T], bf16, tag=f"rca{off}", name=f"rca{off}")
            nc.vector.memset(t_, 0.0)
            rca[off] = t_

        def warmup(n, tagp):
            for w in range(n):
                wps = PSM.tile([128, T], f32, tag="sc", name=f"{tagp}{w}")
                nc.tensor.matmul(
                    wps[:, 0:512], dummy_sb[:, 0:128], dummy_sb, start=True, stop=True
                )
        # vt tiles: per s-chunk [128, 8*65]; cols 65h..65h+63 = head h v
        # channels, col 65h+64 = ones -> U row 64 = softmax denominator
        vt = []
        for si in range(NS):
            t_ = P.tile([128, 8 * 65], bf16, tag=f"vt{si}", name=f"vt{si}")
            v3 = t_[:].rearrange("p (h f) -> p h f", f=65)
            nc.vector.memset(v3[:, :, 64:65], 1.0)
            vt.append(t_)

        # ---- phase 1: adaLN modulation + groupnorm stats ----
        warmup(WARMUP_PRE, "wp")  # keep PE busy from t=0 (HAM clock gate)
        # vb_bc: [128, 512] f32 = v-bias broadcast across partitions (PE
        # rank-1); AFTER the dep-free warmup so the PE isn't blocked at t=0
        ones1 = P.tile([1, 128], bf16, tag="ones1")
        nc.vector.memset(ones1, 1.0)
        vbb_ps = PSM.tile([128, 512], f32, tag="sc", name="vbbps")
        nc.tensor.matmul(vbb_ps, ones1, vb_sb[0:1, :], start=True, stop=True)
        vb_bc = P.tile([128, 512], f32, tag="vb_bc")
        nc.vector.tensor_copy(vb_bc, vbb_ps)
        sg_sb = P.tile([128, 4], f32, tag="sg")
        silu_sb = P.tile([128, 4], f8, tag="silu")
        nc.scalar.activation(sg_sb, emb_sb, AF.Sigmoid)
        nc.vector.tensor_mul(silu_sb, emb_sb, sg_sb)
        # preload the exp table set on ACT while idle
        exwarm = P.tile([1, 4], f32, tag="exwarm")
        nc.scalar.activation(exwarm, sg_sb[0:1, 0:4], AF.Exp)
        # mod^T = silu^T @ ada_wT as [1, 1536], then DRAM-bounce to [128, 12]
        mrow = P.tile([1, 3 * C], f32, tag="mrow")
        for oc in range(3):
            mps = PSM.tile([1, 512], f32, tag="sc", name=f"mps{oc}")
            for j in range(4):
                nc.tensor.matmul(
                    mps,
                    silu_sb[:, j : j + 1],
                    aw[j][:, 512 * oc : 512 * (oc + 1)],
                    start=(j == 0),
                    stop=(j == 3),
                )
            nc.vector.tensor_scalar_mul(
                mrow[:, 512 * oc : 512 * (oc + 1)], mps, 1.0 / WSCALE
            )
        mod_sb = P.tile([128, 12], f32, tag="mod")
        modp_sb = P.tile([128, 12], f32, tag="modp")
        DP = ctx.enter_context(tc.tile_pool(name="dramp", bufs=1, space="DRAM"))
        mod_scr = DP.tile([1, 3 * C], f32, tag="modscr")
        nc.sync.dma_start(out=mod_scr, in_=mrow)
        nc.sync.dma_start(
            out=modp_sb, in_=mod_scr[0, :].rearrange("(f p) -> p f", p=128)
        )
        nc.vector.tensor_add(mod_sb, modp_sb, ab_sb)

        warmup(WARMUP_A, "wa")  # cover the x-DMA + stats window
        mv = []
        for i in range(NC_):
            st6 = P.tile([128, 2, 6], f32, tag=f"st6{i}")
            xv = xf[i][:].rearrange("p (s f) -> p s f", f=512)
            for si in range(2):
                nc.vector.bn_stats(st6[:, si, :], xv[:, si, :])
            mv_i = P.tile([128, 2], f32, tag=f"mv{i}")
            nc.vector.bn_aggr(mv_i, st6)
            # E2 = var + mu^2 into col 1
            tm = P.tile([128, 1], f32, tag=f"tmu{i}")
            nc.vector.tensor_mul(tm, mv_i[:, 0:1], mv_i[:, 0:1])
            nc.vector.tensor_add(mv_i[:, 1:2], mv_i[:, 1:2], tm)
            mv.append(mv_i)

        stats8_ps = PSM.tile([8, 8], f32, tag="sc", name="stats8")
        for i in range(NC_):
            nc.tensor.matmul(
                stats8_ps[:, 2 * i : 2 * i + 2], gind_sb, mv[i], start=True, stop=True
            )
        s8 = P.tile([8, 8], f32, tag="s8")
        nc.vector.tensor_copy(s8, stats8_ps)
        musq8 = P.tile([8, 4], f32, tag="musq8")
        var8 = P.tile([8, 4], f32, tag="var8")
        sd8 = P.tile([8, 4], f32, tag="sd8")
        rstd8 = P.tile([8, 4], f32, tag="rstd8")
        for i in range(NC_):
            nc.vector.tensor_mul(
                musq8[:, i : i + 1], s8[:, 2 * i : 2 * i + 1], s8[:, 2 * i : 2 * i + 1]
            )
            nc.vector.tensor_sub(
                var8[:, i : i + 1], s8[:, 2 * i + 1 : 2 * i + 2], musq8[:, i : i + 1]
            )
        eps8 = P.tile([8, 1], f32, tag="eps8")
        nc.vector.memset(eps8, EPS)
        nc.scalar.activation(sd8, var8, AF.Sqrt, bias=eps8)
        nc.vector.reciprocal(rstd8, sd8)

        xm8p = [
            P.tile([128, 2 * T], f8, tag=f"xm8p{jj}", name=f"xm8p{jj}")
            for jj in range(2)
        ]
        for i in range(NC_):
            statbc = PSM.tile([128, 2], f32, tag="sc", name=f"statbc{i}")
            nc.tensor.matmul(
                statbc[:, 0:1], gindT_sb, s8[:, 2 * i : 2 * i + 1], start=True, stop=True
            )
            nc.tensor.matmul(
                statbc[:, 1:2], gindT_sb, rstd8[:, i : i + 1], start=True, stop=True
            )
            s1p = P.tile([128, 1], f32, tag=f"s1p{i}")
            A_i = P.tile([128, 1], f32, tag=f"A{i}")
            B_i = P.tile([128, 1], f32, tag=f"B{i}")
            tm2 = P.tile([128, 1], f32, tag=f"tm2{i}")
            nc.vector.tensor_scalar_add(s1p, mod_sb[:, 4 + i : 5 + i], 1.0)
            nc.vector.tensor_mul(A_i, statbc[:, 1:2], s1p)
            nc.vector.tensor_mul(tm2, statbc[:, 0:1], A_i)
            nc.vector.tensor_sub(B_i, mod_sb[:, i : i + 1], tm2)
            xm_i = xm8p[i // 2][:, T * (i % 2) : T * (i % 2 + 1)]
            if i % 2 == 0:
                nc.scalar.activation(xm_i, xf[i], AF.Identity, bias=B_i, scale=A_i)
            else:
                nc.vector.tensor_scalar(
                    xm_i, xf[i][:], A_i[:], B_i[:], ALU.mult, ALU.add
                )

        warmup(WARMUP_B, "wb")  # bridge the xm-activation latency into qkv

        # ---- phase 2: q,k [cout, T] + vT [s, c] ----
        # qk_sb[0..3] = q chunks, qk_sb[4..7] = k chunks (type-major perm order)
        qk_sb = [
            P.tile([128, T], bf16, tag=f"qk{m}", name=f"qk{m}") for m in range(8)
        ]
        # a8p: DoubleRow pair tiles for proj rhs: [:, :1024] = head-pair 2jj,
        # [:, 1024:] = head-pair 2jj+1 (fp8, written by the normalize muls)
        a8p = [
            P.tile([128, 2 * T], f8, tag=f"a8p{jj}", name=f"a8p{jj}")
            for jj in range(2)
        ]
        DR = mybir.MatmulPerfMode.DoubleRow

        def qw3(jj, lo, hi):
            return qw8p[jj][:].rearrange("p (two f) -> p two f", two=2)[:, :, lo:hi]

        def xm3(jj, lo, hi):
            return xm8p[jj][:].rearrange("p (two f) -> p two f", two=2)[:, :, lo:hi]

        for blk in range(4):
            for m in (blk, 4 + blk):  # q chunk then k chunk
                ps = PSM.tile([128, T], f32, tag="sc", name=f"qkps{m}")
                for jj in range(2):
                    for t in range(NT):
                        nc.tensor.matmul(
                            ps[:, 512 * t : 512 * (t + 1)],
                            qw3(jj, 128 * m, 128 * (m + 1)),
                            xm3(jj, 512 * t, 512 * (t + 1)),
                            start=(jj == 0),
                            stop=(jj == 1),
                            perf_mode=DR,
                        )
                # eviction on ACT (idle during phase 2): out = ps/WSCALE + qb
                nc.scalar.activation(
                    qk_sb[m][:], ps, AF.Identity,
                    bias=qb_sb[:, m : m + 1], scale=1.0 / WSCALE,
                )
            for si in (2 * blk, 2 * blk + 1):  # vT chunks
                vps = PSM.tile([128, T], f32, tag="sc", name=f"vtps{si}")
                for jj in range(2):
                    nc.tensor.matmul(
                        vps[:, 0:512],
                        xm3(jj, 128 * si, 128 * (si + 1)),
                        qw3(jj, 1024, 1536),
                        start=(jj == 0),
                        stop=(jj == 1),
                        perf_mode=DR,
                    )
                # vt = vps/WSCALE + vb (strided into the per-head 65-blocks)
                nc.vector.scalar_tensor_tensor(
                    vt[si][:].rearrange("p (h f) -> p h f", f=65)[:, :, 0:64],
                    vps[:, 0:512].rearrange("p (h f) -> p h f", f=64),
                    1.0 / WSCALE,
                    vb_bc[:].rearrange("p (h f) -> p h f", f=64),
                    ALU.mult,
                    ALU.add,
                )

        # ---- phase 3+4: attention, flat cross-pair software pipeline ----
        # Global slot g = 8*pair + si. Per slot: PV(g-4) then scores(g);
        # exp head A on ACT, head B on DVE (Schraudolph). At each pair
        # boundary: ONE ACT copy of U[0:65] (a_unnorm + denom row) to an
        # SBUF stage tile releases the U banks; then PE e64-broadcast of
        # the denom row -> DVE reciprocal -> multiply stage*recip into
        # a_sb (head A on gpsimd, head B on DVE with shifted output).
        stage = {}
        for off in (0, 64):
            stage[off] = P.tile([65, T], bf16, tag=f"stage{off}", name=f"stage{off}")
        SLOTS = 4 * NS
        Upair = {}
        ex_tiles = {}

        def emit_scores(g):
            p, si = divmod(g, NS)
            heads = (2 * p, 2 * p + 1)
            if si == 0:
                Upair[p] = {
                    h: PSU.tile([65, T], f32, tag="u", name=f"u{p}_{h}")
                    for h in heads
                }
            for h in heads:
                off = 64 * (h % 2)
                q_ap = qk_sb[p][off : off + 64, :]
                k_ap = qk_sb[4 + p][off : off + 64, :]
                sc = PSM.tile([128, T], f32, tag="sc", name=f"sc{p}_{si}_{h}")
                for t in range(NT):
                    nc.tensor.matmul(
                        sc[:, 512 * t : 512 * (t + 1)],
                        k_ap[:, 128 * si : 128 * (si + 1)],
                        q_ap[:, 512 * t : 512 * (t + 1)],
                        start=True,
                        stop=True,
                        tile_position=(off, 0),
                    )
                if h == heads[0]:
                    ex = EXPP.tile([128, T], bf16, tag="ex")
                    nc.scalar.activation(ex, sc, AF.Exp, scale=0.125)
                    ex_tiles[(h, si)] = ex
                elif EXP_SPLIT:
                    exb = EXPI.tile([128, T], i16, tag="exi")
                    nc.vector.tensor_scalar(
                        exb[:], sc, EXP_A, EXP_B, ALU.mult, ALU.add
                    )
                    ex_tiles[(h, si)] = exb[:].bitcast(bf16)
                else:
                    exb = EXPP.tile([128, T], bf16, tag="ex")
                    nc.scalar.activation(exb, sc, AF.Exp, scale=0.125)
                    ex_tiles[(h, si)] = exb

        def emit_pv(g):
            p, si = divmod(g, NS)
            for h in (2 * p, 2 * p + 1):
                ex = ex_tiles.pop((h, si))
                for t in range(NT):
                    nc.tensor.matmul(
                        Upair[p][h][:, 512 * t : 512 * (t + 1)],
                        vt[si][:, 65 * h : 65 * h + 65],
                        ex[:, 512 * t : 512 * (t + 1)],
                        start=(si == 0),
                        stop=(si == NS - 1),
                    )

        def emit_stage_copies(p, split=False):
            # one copy per head: U[0:65] -> stage (bf16); releases U banks.
            # Tail pair: head B's copy on DVE so the two copies run in parallel.
            for h in (2 * p, 2 * p + 1):
                off = 64 * (h % 2)
                if split and off == 64:
                    nc.vector.tensor_copy(stage[off][:, :], Upair[p][h][0:65, :])
                else:
                    nc.scalar.copy(stage[off][:, :], Upair[p][h][0:65, :])
            del Upair[p]

        bcs = {}

        def emit_bc_recip(p):
            # PE: broadcast denom row 64 across 64 partitions; DVE: reciprocal
            for h in (2 * p, 2 * p + 1):
                off = 64 * (h % 2)
                bc = PSM.tile([64, T], f32, tag="sc", name=f"bc{p}_{h}")
                for t in range(NT):
                    nc.tensor.matmul(
                        bc[:, 512 * t : 512 * (t + 1)],
                        e64[:, 0:64],
                        stage[off][0:65, 512 * t : 512 * (t + 1)],
                        start=True,
                        stop=True,
                    )
                rbs = ANP.tile([64, T], f32, tag=f"rbs{off}")
                nc.vector.reciprocal_approx_fast(out=rbs[:], in_=bc[:])
                bcs[(p, h)] = rbs

        def emit_muls(p):
            ha, hb = 2 * p, 2 * p + 1
            dst = a8p[p // 2][:, T * (p % 2) : T * (p % 2 + 1)]
            # head A: gpsimd (all base 0); head B: DVE shifted output
            nc.gpsimd.tensor_mul(
                dst[0:64, :], stage[0][0:64, :], bcs.pop((p, ha))[:]
            )
            nc.vector.tensor_mul(
                dst[64:128, :], stage[64][0:64, :], bcs.pop((p, hb))[:]
            )

        for g in range(SLOTS):
            p, si = divmod(g, NS)
            if si == 4 and p >= 1:
                emit_bc_recip(p - 1)
            if g >= 4:
                emit_pv(g - 4)
            emit_scores(g)
            if si == 3 and p >= 1:
                emit_stage_copies(p - 1)
            if si == 5 and p >= 1:
                emit_muls(p - 1)
        for g in range(SLOTS, SLOTS + 4):
            emit_pv(g - 4)
        emit_stage_copies(3, split=True)

        # ---- phase 5: proj (+bias via rank-1) + fused gated residual ----
        # partials (j=0..2) overlap the last pair normalize; j=3 + bias
        # lands once a_sb[3] is ready.
        proj_ps = {}
        gate64 = P.tile([128, 4], f32, tag="gate64")
        nc.vector.tensor_scalar_mul(gate64, mod_sb[:, 8:12], 1.0 / WSCALE)

        def pw3(jj, lo, hi):
            return pw8p[jj][:].rearrange("p (two f) -> p two f", two=2)[:, :, lo:hi]

        def a3(jj, lo, hi):
            return a8p[jj][:].rearrange("p (two f) -> p two f", two=2)[:, :, lo:hi]

        def proj_partial(m):
            # jj=0 (pairs 0,1) as DoubleRow + pair 2 as plain fp8: everything
            # that doesn't need the last pair's normalize
            ps = PSM.tile([128, T], f32, tag="sc", name=f"projps{m}")
            proj_ps[m] = ps
            for t in range(NT):
                nc.tensor.matmul(
                    ps[:, 512 * t : 512 * (t + 1)],
                    pw3(0, 128 * m, 128 * (m + 1)),
                    a3(0, 512 * t, 512 * (t + 1)),
                    start=True,
                    stop=False,
                    perf_mode=DR,
                )
            for t in range(NT):
                nc.tensor.matmul(
                    ps[:, 512 * t : 512 * (t + 1)],
                    pw8p[1][:, 128 * m : 128 * (m + 1)],
                    a8p[1][:, 512 * t : 512 * (t + 1)],
                    start=False,
                    stop=False,
                )

        def proj_finish(m):
            ps = proj_ps.pop(m)
            for t in range(NT):
                nc.tensor.matmul(
                    ps[:, 512 * t : 512 * (t + 1)],
                    pw8p[1][:, 512 + 128 * m : 512 + 128 * (m + 1)],
                    a8p[1][:, T + 512 * t : T + 512 * (t + 1)],
                    start=False,
                    stop=False,
                )
            for t in range(NT):
                nc.tensor.matmul(
                    ps[:, 512 * t : 512 * (t + 1)],
                    pbr_sb[0:1, 128 * m : 128 * (m + 1)],
                    ones_row[0:1, 512 * t : 512 * (t + 1)],
                    start=False,
                    stop=True,
                )
            # xf = xf + (gate/WSCALE) * ps   (ps = WSCALE*(proj + pb))
            nc.vector.scalar_tensor_tensor(
                xf[m][:], ps, gate64[:, m : m + 1], xf[m][:],
                ALU.mult, ALU.add,
            )
            nc.sync.dma_start(out=out_d.ap()[128 * m : 128 * (m + 1), :], in_=xf[m])

        emit_bc_recip(3)
        proj_partial(0)
        proj_partial(1)
        emit_muls(3)
        proj_finish(0)
        proj_partial(2)
        proj_finish(1)
        proj_partial(3)
        proj_finish(2)
        proj_finish(3)

    nc.compile()
    return nc


_PROGRAM = None
LAST_RESULTS = None


def _get_program():
    global _PROGRAM
    if _PROGRAM is None:
        _PROGRAM = _build_program()
    return _PROGRAM


def kernel(x, emb, qkv_w, qkv_b, ada_w, ada_b, proj_w, proj_b, _trace=False):
    global LAST_RESULTS
    import ml_dtypes

    nc = _get_program()

    x = np.asarray(x, np.float32)
    emb = np.asarray(emb, np.float32)
    perm = _perm()
    bf = ml_dtypes.bfloat16
    f8n = ml_dtypes.float8_e4m3

    def to_f8(a):
        return np.ascontiguousarray(np.clip(a * 64.0, -240.0, 240.0).astype(f8n))

    qkv_wT = to_f8(np.asarray(qkv_w, np.float32)[perm, :].T)
    qkv_b_p = np.ascontiguousarray(np.asarray(qkv_b, np.float32)[perm])
    vbrow = np.ascontiguousarray(qkv_b_p[1024:].astype(bf).reshape(1, C))
    ada_wT = to_f8(np.asarray(ada_w, np.float32).T)
    ada_b = np.ascontiguousarray(np.asarray(ada_b, np.float32))
    proj_wT = to_f8(np.asarray(proj_w, np.float32).T)
    pbrow = to_f8(np.asarray(proj_b, np.float32).reshape(1, C))

    gind = np.repeat(np.eye(8, dtype=np.float32), 16, axis=0) / 16.0  # [128, 8]
    gindT = np.ascontiguousarray(np.repeat(np.eye(8, dtype=np.float32), 16, axis=0).T)

    in_maps = []
    for b in range(B):
        in_maps.append(
            {
                "x": np.ascontiguousarray(x[b].reshape(C, T)),
                "emb": np.ascontiguousarray(emb[b]),
                "qkv_wT": qkv_wT,
                "qkv_b": qkv_b_p,
                "vbrow": vbrow,
                "ada_wT": ada_wT,
                "ada_b": ada_b,
                "proj_wT": proj_wT,
                "pbrow": pbrow,
                "gind": gind,
                "gindT": gindT,
            }
        )

    res = run_bass_kernel_spmd(nc, in_maps, list(range(8)), trace=_trace)
    LAST_RESULTS = res
    out = np.stack([res.results[b]["out"] for b in range(B)], axis=0)
    return np.ascontiguousarray(out.reshape(B, C, HH, WW).astype(np.float32))


# revision 47
# speedup vs baseline: 1.1021x; 1.1021x over previous
"""Trainium2 Bass kernel for nn_AttentionBlock (adaLN-modulated GroupNorm attention).

Sharding: data-parallel over batch B=8 -> one batch per NeuronCore (8 cores).
Each core runs the full block for its batch:
  groupnorm(32 groups) -> adaLN modulate -> qkv matmul -> 8-head attention
  (softmax over keys) -> proj matmul -> gated residual.

Final design (v6): restructured for PE density / HAM clock-gate warmth:
  - q, k computed as [cout, T] tiles (type-major permuted channel order) so
    head h's q/k live at partition offset 64*(h%2) of tile h//2.
  - v computed TRANSPOSED directly by the qkv matmul (lhsT = xm chunk,
    rhs = v-weight columns) -> vt tiles [128 s, 8*65]: per head 65 cols =
    [v channels | ones], so PV's U output carries the softmax denominator
    in partition row 64 (no ones appended via copies, no PE transposes).
  - fp8 (e4m3) DoubleRow matmuls for qkv, vT and proj (weights pre-scaled
    x64 to clear the fp8 normal floor; rescaled for free in the evictions);
    scores and PV stay bf16. Halves those matmul cycles AND the weight DMA.
  - flat cross-pair software pipeline: PV runs 4 slots behind scores, all
    pair-boundary work (stage copy / broadcast / reciprocal / normalize
    muls) is spread across fixed slot offsets so the PE never bunches.
  - exp split: head A of each pair on ScalarE (exact exp), head B on DVE
    via the Schraudolph bit trick (bf16 bits ~= int16(A*s + B)); the
    uniform ~0.3% scale error cancels in softmax normalization.
  - normalize: ONE ACT copy of U[0:65] to an SBUF stage tile (releases the
    U PSUM banks early) -> PE rank-1 e64 broadcast of the denom row -> DVE
    fast reciprocal -> multiply stage*recip into the fp8 a-tiles (head A on
    gpsimd, head B on DVE using its partition-shifted output).
  - proj bias folded into the proj matmul (rank-1: lhsT=pb, rhs=ones);
    gated residual fused in one DVE scalar_tensor_tensor per chunk.
  - PE warmup burst of dummy matmuls before qkv to lift the HAM clock gate.

Attention matmuls run in bf16, qkv/vT/proj in fp8 DoubleRow (fp32 PSUM
accumulation everywhere); groupnorm statistics and the residual stay fp32.
"""

import numpy as np

import concourse.bass as bass
import concourse.tile as tile
from concourse import bacc, mybir
from concourse.bass_utils import run_bass_kernel_spmd

AF = mybir.ActivationFunctionType
ALU = mybir.AluOpType
f32 = mybir.dt.float32
bf16 = mybir.dt.bfloat16
i16 = mybir.dt.int16
f8 = mybir.dt.float8e4
WSCALE = 64.0        # fp8 weights pre-scaled by 64 (dodges fp8 normal floor)

B, C, HH, WW, E = 8, 512, 32, 32, 512
HEADS, G = 8, 32
T = HH * WW          # 1024
CH = C // HEADS      # 64
NC_ = C // 128       # 4 channel chunks
NT = T // 512        # 2 t-chunks of 512
NS = T // 128        # 8 s-chunks of 128
EPS = 1e-5
WARMUP_PRE = 12      # PE warmup matmuls at t=0 (HAM clock-gate lift)
WARMUP_A = 40        # warmup covering the x-DMA/stats window
WARMUP_B = 24        # warmup bridging xm latency into qkv
EXP_SPLIT = True     # head B exp on DVE (Schraudolph); False -> all on ACT
# Schraudolph: bf16 bits of exp(0.125*s) ~= int16(EXP_A*s + EXP_B)
EXP_A = 128.0 * 0.125 / float(np.log(2.0))
EXP_B = 16256.0 - 128.0 * 0.043


def _perm():
    """new[512*ty + 64*h + r] = orig[192*h + 64*ty + r] (head-major -> type-major)."""
    p = np.empty(3 * C, np.int64)
    for h in range(HEADS):
        for ty in range(3):
            p[512 * ty + 64 * h : 512 * ty + 64 * h + 64] = (
                192 * h + 64 * ty + np.arange(64)
            )
    return p


def _build_program():
    nc = bacc.Bacc("TRN2", target_bir_lowering=False, debug=False, num_devices=8)

    # ---- DRAM parameters (per-core shards; weights replicated, bf16) ----
    x_d = nc.declare_dram_parameter("x", [C, T], f32, isOutput=False)
    emb_d = nc.declare_dram_parameter("emb", [E], f32, isOutput=False)
    qw_d = nc.declare_dram_parameter("qkv_wT", [C, 3 * C], f8, isOutput=False)
    qb_d = nc.declare_dram_parameter("qkv_b", [3 * C], f32, isOutput=False)
    vb_d = nc.declare_dram_parameter("vbrow", [1, C], bf16, isOutput=False)
    aw_d = nc.declare_dram_parameter("ada_wT", [E, 3 * C], f8, isOutput=False)
    ab_d = nc.declare_dram_parameter("ada_b", [3 * C], f32, isOutput=False)
    pw_d = nc.declare_dram_parameter("proj_wT", [C, C], f8, isOutput=False)
    pb_d = nc.declare_dram_parameter("pbrow", [1, C], f8, isOutput=False)
    gind_d = nc.declare_dram_parameter("gind", [128, 8], f32, isOutput=False)
    gindT_d = nc.declare_dram_parameter("gindT", [8, 128], f32, isOutput=False)
    out_d = nc.declare_dram_parameter("out", [C, T], f32, isOutput=True)

    from contextlib import ExitStack

    with tile.TileContext(nc) as tc, ExitStack() as ctx:
        ctx.enter_context(
            nc.allow_low_precision(reason="bf16 matmul inputs; fp32 accumulate")
        )
        P = ctx.enter_context(tc.tile_pool(name="persist", bufs=1))
        # PSUM: "sc" slots [128,1024] f32 = 2 banks x 2 bufs; "u" = 2 banks x 2
        PSM = ctx.enter_context(tc.tile_pool(name="psm", bufs=2, space="PSUM"))
        PSU = ctx.enter_context(tc.tile_pool(name="psu", bufs=2, space="PSUM"))
        EXPP = ctx.enter_context(tc.tile_pool(name="expp", bufs=6))
        EXPI = ctx.enter_context(tc.tile_pool(name="expi", bufs=6))
        ANP = ctx.enter_context(tc.tile_pool(name="anp", bufs=2))

        # ---- persistent SBUF tiles + input DMAs (ordered by need time) ----
        gind_sb = P.tile([128, 8], f32, tag="gind")
        gindT_sb = P.tile([8, 128], f32, tag="gindT")
        emb_sb = P.tile([128, 4], f32, tag="emb")
        qb_sb = P.tile([128, 12], f32, tag="qb")
        ab_sb = P.tile([128, 12], f32, tag="ab")
        vb_sb = P.tile([1, C], bf16, tag="vb")
        pbr_sb = P.tile([1, C], f8, tag="pbr")

        nc.sync.dma_start(out=gind_sb, in_=gind_d.ap())
        nc.sync.dma_start(out=gindT_sb, in_=gindT_d.ap())
        nc.sync.dma_start(out=emb_sb, in_=emb_d.ap().rearrange("(f p) -> p f", p=128))
        nc.sync.dma_start(out=qb_sb, in_=qb_d.ap().rearrange("(f p) -> p f", p=128))
        nc.sync.dma_start(out=ab_sb, in_=ab_d.ap().rearrange("(f p) -> p f", p=128))
        nc.sync.dma_start(out=vb_sb, in_=vb_d.ap())
        nc.sync.dma_start(out=pbr_sb, in_=pb_d.ap())

        xf = []
        for i in range(NC_):
            t_ = P.tile([128, T], f32, tag=f"xf{i}", name=f"xf{i}")
            for hh in range(2):
                nc.sync.dma_start(
                    out=t_[:, 512 * hh : 512 * (hh + 1)],
                    in_=x_d.ap()[128 * i : 128 * (i + 1), 512 * hh : 512 * (hh + 1)],
                )
            xf.append(t_)
        aw = []
        for j in range(4):
            t_ = P.tile([128, 3 * C], f8, tag=f"aw{j}", name=f"aw{j}")
            nc.sync.dma_start(out=t_, in_=aw_d.ap()[128 * j : 128 * (j + 1), :])
            aw.append(t_)
        # qw/pw as DoubleRow pair tiles: [:, :1536] = chunk 2jj, [:, 1536:] = 2jj+1
        qw8p = []
        for jj in range(2):
            t_ = P.tile([128, 2 * 3 * C], f8, tag=f"qw8p{jj}", name=f"qw8p{jj}")
            for i2 in range(2):
                j = 2 * jj + i2
                nc.sync.dma_start(
                    out=t_[:, 1536 * i2 : 1536 * (i2 + 1)],
                    in_=qw_d.ap()[128 * j : 128 * (j + 1), :],
                )
            qw8p.append(t_)
        pw8p = []
        for jj in range(2):
            t_ = P.tile([128, 2 * C], f8, tag=f"pw8p{jj}", name=f"pw8p{jj}")
            for i2 in range(2):
                j = 2 * jj + i2
                nc.sync.dma_start(
                    out=t_[:, 512 * i2 : 512 * (i2 + 1)],
                    in_=pw_d.ap()[128 * j : 128 * (j + 1), :],
                )
            pw8p.append(t_)

        # small constants built on-chip
        ones_row = P.tile([1, T], f8, tag="ones_row")
        nc.vector.memset(ones_row, 1.0)
        dummy_sb = P.tile([128, 512], bf16, tag="dummy")
        nc.vector.memset(dummy_sb, 0.0)
        # vb_bc: [128, 512] f32 = v-bias broadcast across partitions (PE rank-1)
        ones1 = P.tile([1, 128], bf16, tag="ones1")
        nc.vector.memset(ones1, 1.0)
        vbb_ps = PSM.tile([128, 512], f32, tag="sc", name="vbbps")
        nc.tensor.matmul(vbb_ps, ones1, vb_sb[0:1, :], start=True, stop=True)
        vb_bc = P.tile([128, 512], f32, tag="vb_bc")
        nc.vector.tensor_copy(vb_bc, vbb_ps)
        # e64: [65, 64] with row 64 = ones -> PE broadcast of denominator row
        e64 = P.tile([65, 64], bf16, tag="e64")
        nc.vector.memset(e64, 0.0)
        nc.vector.memset(e64[64:65, :], 1.0)
        # rca: per-head-slot staging for the denominator row (row 64); rows
        # 0:64 stay zero so the e64 broadcast matmul sees no garbage
        rca = {}
        for off in (0, 64):
            t_ = P.tile([65, T], bf16, tag=f"rca{off}", name=f"rca{off}")
            nc.vector.memset(t_, 0.0)
            rca[off] = t_

        def warmup(n, tagp):
            for w in range(n):
                wps = PSM.tile([128, T], f32, tag="sc", name=f"{tagp}{w}")
                nc.tensor.matmul(
                    wps[:, 0:512], dummy_sb[:, 0:128], dummy_sb, start=True, stop=True
                )
        # vt tiles: per s-chunk [128, 8*65]; cols 65h..65h+63 = head h v
        # channels, col 65h+64 = ones -> U row 64 = softmax denominator
        vt = []
        for si in range(NS):
            t_ = P.tile([128, 8 * 65], bf16, tag=f"vt{si}", name=f"vt{si}")
            v3 = t_[:].rearrange("p (h f) -> p h f", f=65)
            nc.vector.memset(v3[:, :, 64:65], 1.0)
            vt.append(t_)

        # ---- phase 1: adaLN modulation + groupnorm stats ----
        warmup(WARMUP_PRE, "wp")  # keep PE busy from t=0 (HAM clock gate)
        sg_sb = P.tile([128, 4], f32, tag="sg")
        silu_sb = P.tile([128, 4], f8, tag="silu")
        nc.scalar.activation(sg_sb, emb_sb, AF.Sigmoid)
        nc.vector.tensor_mul(silu_sb, emb_sb, sg_sb)
        # preload the exp table set on ACT while idle
        exwarm = P.tile([1, 4], f32, tag="exwarm")
        nc.scalar.activation(exwarm, sg_sb[0:1, 0:4], AF.Exp)
        # mod^T = silu^T @ ada_wT as [1, 1536], then DRAM-bounce to [128, 12]
        mrow = P.tile([1, 3 * C], f32, tag="mrow")
        for oc in range(3):
            mps = PSM.tile([1, 512], f32, tag="sc", name=f"mps{oc}")
            for j in range(4):
                nc.tensor.matmul(
                    mps,
                    silu_sb[:, j : j + 1],
                    aw[j][:, 512 * oc : 512 * (oc + 1)],
                    start=(j == 0),
                    stop=(j == 3),
                )
            nc.vector.tensor_scalar_mul(
                mrow[:, 512 * oc : 512 * (oc + 1)], mps, 1.0 / WSCALE
            )
        mod_sb = P.tile([128, 12], f32, tag="mod")
        modp_sb = P.tile([128, 12], f32, tag="modp")
        DP = ctx.enter_context(tc.tile_pool(name="dramp", bufs=1, space="DRAM"))
        mod_scr = DP.tile([1, 3 * C], f32, tag="modscr")
        nc.sync.dma_start(out=mod_scr, in_=mrow)
        nc.sync.dma_start(
            out=modp_sb, in_=mod_scr[0, :].rearrange("(f p) -> p f", p=128)
        )
        nc.vector.tensor_add(mod_sb, modp_sb, ab_sb)

        warmup(WARMUP_A, "wa")  # cover the x-DMA + stats window
        mv = []
        for i in range(NC_):
            st6 = P.tile([128, 2, 6], f32, tag=f"st6{i}")
            xv = xf[i][:].rearrange("p (s f) -> p s f", f=512)
            for si in range(2):
                nc.vector.bn_stats(st6[:, si, :], xv[:, si, :])
            mv_i = P.tile([128, 2], f32, tag=f"mv{i}")
            nc.vector.bn_aggr(mv_i, st6)
            # E2 = var + mu^2 into col 1
            tm = P.tile([128, 1], f32, tag=f"tmu{i}")
            nc.vector.tensor_mul(tm, mv_i[:, 0:1], mv_i[:, 0:1])
            nc.vector.tensor_add(mv_i[:, 1:2], mv_i[:, 1:2], tm)
            mv.append(mv_i)

        stats8_ps = PSM.tile([8, 8], f32, tag="sc", name="stats8")
        for i in range(NC_):
            nc.tensor.matmul(
                stats8_ps[:, 2 * i : 2 * i + 2], gind_sb, mv[i], start=True, stop=True
            )
        s8 = P.tile([8, 8], f32, tag="s8")
        nc.vector.tensor_copy(s8, stats8_ps)
        musq8 = P.tile([8, 4], f32, tag="musq8")
        var8 = P.tile([8, 4], f32, tag="var8")
        sd8 = P.tile([8, 4], f32, tag="sd8")
        rstd8 = P.tile([8, 4], f32, tag="rstd8")
        for i in range(NC_):
            nc.vector.tensor_mul(
                musq8[:, i : i + 1], s8[:, 2 * i : 2 * i + 1], s8[:, 2 * i : 2 * i + 1]
            )
            nc.vector.tensor_sub(
                var8[:, i : i + 1], s8[:, 2 * i + 1 : 2 * i + 2], musq8[:, i : i + 1]
            )
        eps8 = P.tile([8, 1], f32, tag="eps8")
        nc.vector.memset(eps8, EPS)
        nc.scalar.activation(sd8, var8, AF.Sqrt, bias=eps8)
        nc.vector.reciprocal(rstd8, sd8)

        xm8p = [
            P.tile([128, 2 * T], f8, tag=f"xm8p{jj}", name=f"xm8p{jj}")
            for jj in range(2)
        ]
        for i in range(NC_):
            statbc = PSM.tile([128, 2], f32, tag="sc", name=f"statbc{i}")
            nc.tensor.matmul(
                statbc[:, 0:1], gindT_sb, s8[:, 2 * i : 2 * i + 1], start=True, stop=True
            )
            nc.tensor.matmul(
                statbc[:, 1:2], gindT_sb, rstd8[:, i : i + 1], start=True, stop=True
            )
            s1p = P.tile([128, 1], f32, tag=f"s1p{i}")
            A_i = P.tile([128, 1], f32, tag=f"A{i}")
            B_i = P.tile([128, 1], f32, tag=f"B{i}")
            tm2 = P.tile([128, 1], f32, tag=f"tm2{i}")
            nc.vector.tensor_scalar_add(s1p, mod_sb[:, 4 + i : 5 + i], 1.0)
            nc.vector.tensor_mul(A_i, statbc[:, 1:2], s1p)
            nc.vector.tensor_mul(tm2, statbc[:, 0:1], A_i)
            nc.vector.tensor_sub(B_i, mod_sb[:, i : i + 1], tm2)
            xm_i = xm8p[i // 2][:, T * (i % 2) : T * (i % 2 + 1)]
            if i % 2 == 0:
                nc.scalar.activation(xm_i, xf[i], AF.Identity, bias=B_i, scale=A_i)
            else:
                nc.vector.tensor_scalar(
                    xm_i, xf[i][:], A_i[:], B_i[:], ALU.mult, ALU.add
                )

        warmup(WARMUP_B, "wb")  # bridge the xm-activation latency into qkv

        # ---- phase 2: q,k [cout, T] + vT [s, c] ----
        # qk_sb[0..3] = q chunks, qk_sb[4..7] = k chunks (type-major perm order)
        qk_sb = [
            P.tile([128, T], bf16, tag=f"qk{m}", name=f"qk{m}") for m in range(8)
        ]
        # a8p: DoubleRow pair tiles for proj rhs: [:, :1024] = head-pair 2jj,
        # [:, 1024:] = head-pair 2jj+1 (fp8, written by the normalize muls)
        a8p = [
            P.tile([128, 2 * T], f8, tag=f"a8p{jj}", name=f"a8p{jj}")
            for jj in range(2)
        ]
        DR = mybir.MatmulPerfMode.DoubleRow

        def qw3(jj, lo, hi):
            return qw8p[jj][:].rearrange("p (two f) -> p two f", two=2)[:, :, lo:hi]

        def xm3(jj, lo, hi):
            return xm8p[jj][:].rearrange("p (two f) -> p two f", two=2)[:, :, lo:hi]

        for blk in range(4):
            for m in (blk, 4 + blk):  # q chunk then k chunk
                ps = PSM.tile([128, T], f32, tag="sc", name=f"qkps{m}")
                for jj in range(2):
                    for t in range(NT):
                        nc.tensor.matmul(
                            ps[:, 512 * t : 512 * (t + 1)],
                            qw3(jj, 128 * m, 128 * (m + 1)),
                            xm3(jj, 512 * t, 512 * (t + 1)),
                            start=(jj == 0),
                            stop=(jj == 1),
                            perf_mode=DR,
                        )
                # eviction on ACT (idle during phase 2): out = ps/WSCALE + qb
                nc.scalar.activation(
                    qk_sb[m][:], ps, AF.Identity,
                    bias=qb_sb[:, m : m + 1], scale=1.0 / WSCALE,
                )
            for si in (2 * blk, 2 * blk + 1):  # vT chunks
                vps = PSM.tile([128, T], f32, tag="sc", name=f"vtps{si}")
                for jj in range(2):
                    nc.tensor.matmul(
                        vps[:, 0:512],
                        xm3(jj, 128 * si, 128 * (si + 1)),
                        qw3(jj, 1024, 1536),
                        start=(jj == 0),
                        stop=(jj == 1),
                        perf_mode=DR,
                    )
                # vt = vps/WSCALE + vb (strided into the per-head 65-blocks)
                nc.vector.scalar_tensor_tensor(
                    vt[si][:].rearrange("p (h f) -> p h f", f=65)[:, :, 0:64],
                    vps[:, 0:512].rearrange("p (h f) -> p h f", f=64),
                    1.0 / WSCALE,
                    vb_bc[:].rearrange("p (h f) -> p h f", f=64),
                    ALU.mult,
                    ALU.add,
                )

        # ---- phase 3+4: attention, flat cross-pair software pipeline ----
        # Global slot g = 8*pair + si. Per slot: PV(g-4) then scores(g);
        # exp head A on ACT, head B on DVE (Schraudolph). At each pair
        # boundary: ONE ACT copy of U[0:65] (a_unnorm + denom row) to an
        # SBUF stage tile releases the U banks; then PE e64-broadcast of
        # the denom row -> DVE reciprocal -> multiply stage*recip into
        # a_sb (head A on gpsimd, head B on DVE with shifted output).
        stage = {}
        for off in (0, 64):
            stage[off] = P.tile([65, T], bf16, tag=f"stage{off}", name=f"stage{off}")
        SLOTS = 4 * NS
        Upair = {}
        ex_tiles = {}

        def emit_scores(g):
            p, si = divmod(g, NS)
            heads = (2 * p, 2 * p + 1)
            if si == 0:
                Upair[p] = {
                    h: PSU.tile([65, T], f32, tag="u", name=f"u{p}_{h}")
                    for h in heads
                }
            for h in heads:
                off = 64 * (h % 2)
                q_ap = qk_sb[p][off : off + 64, :]
                k_ap = qk_sb[4 + p][off : off + 64, :]
                sc = PSM.tile([128, T], f32, tag="sc", name=f"sc{p}_{si}_{h}")
                for t in range(NT):
                    nc.tensor.matmul(
                        sc[:, 512 * t : 512 * (t + 1)],
                        k_ap[:, 128 * si : 128 * (si + 1)],
                        q_ap[:, 512 * t : 512 * (t + 1)],
                        start=True,
                        stop=True,
                        tile_position=(off, 0),
                    )
                if h == heads[0]:
                    ex = EXPP.tile([128, T], bf16, tag="ex")
                    nc.scalar.activation(ex, sc, AF.Exp, scale=0.125)
                    ex_tiles[(h, si)] = ex
                elif EXP_SPLIT:
                    exb = EXPI.tile([128, T], i16, tag="exi")
                    nc.vector.tensor_scalar(
                        exb[:], sc, EXP_A, EXP_B, ALU.mult, ALU.add
                    )
                    ex_tiles[(h, si)] = exb[:].bitcast(bf16)
                else:
                    exb = EXPP.tile([128, T], bf16, tag="ex")
                    nc.scalar.activation(exb, sc, AF.Exp, scale=0.125)
                    ex_tiles[(h, si)] = exb

        def emit_pv(g):
            p, si = divmod(g, NS)
            for h in (2 * p, 2 * p + 1):
                ex = ex_tiles.pop((h, si))
                for t in range(NT):
                    nc.tensor.matmul(
                        Upair[p][h][:, 512 * t : 512 * (t + 1)],
                        vt[si][:, 65 * h : 65 * h + 65],
                        ex[:, 512 * t : 512 * (t + 1)],
                        start=(si == 0),
                        stop=(si == NS - 1),
                    )

        def emit_stage_copies(p, split=False):
            # one copy per head: U[0:65] -> stage (bf16); releases U banks.
            # Tail pair: head B's copy on DVE so the two copies run in parallel.
            for h in (2 * p, 2 * p + 1):
                off = 64 * (h % 2)
                if split and off == 64:
                    nc.vector.tensor_copy(stage[off][:, :], Upair[p][h][0:65, :])
                else:
                    nc.scalar.copy(stage[off][:, :], Upair[p][h][0:65, :])
            del Upair[p]

        bcs = {}

        def emit_bc_recip(p):
            # PE: broadcast denom row 64 across 64 partitions; DVE: reciprocal
            for h in (2 * p, 2 * p + 1):
                off = 64 * (h % 2)
                bc = PSM.tile([64, T], f32, tag="sc", name=f"bc{p}_{h}")
                for t in range(NT):
                    nc.tensor.matmul(
                        bc[:, 512 * t : 512 * (t + 1)],
                        e64[:, 0:64],
                        stage[off][0:65, 512 * t : 512 * (t + 1)],
                        start=True,
                        stop=True,
                    )
                rbs = ANP.tile([64, T], f32, tag=f"rbs{off}")
                nc.vector.reciprocal_approx_fast(out=rbs[:], in_=bc[:])
                bcs[(p, h)] = rbs

        def emit_muls(p):
            ha, hb = 2 * p, 2 * p + 1
            dst = a8p[p // 2][:, T * (p % 2) : T * (p % 2 + 1)]
            # head A: gpsimd (all base 0); head B: DVE shifted output
            nc.gpsimd.tensor_mul(
                dst[0:64, :], stage[0][0:64, :], bcs.pop((p, ha))[:]
            )
            nc.vector.tensor_mul(
                dst[64:128, :], stage[64][0:64, :], bcs.pop((p, hb))[:]
            )

        for g in range(SLOTS):
            p, si = divmod(g, NS)
            if si == 4 and p >= 1:
                emit_bc_recip(p - 1)
            if g >= 4:
                emit_pv(g - 4)
            emit_scores(g)
            if si == 3 and p >= 1:
                emit_stage_copies(p - 1)
            if si == 5 and p >= 1:
                emit_muls(p - 1)
        for g in range(SLOTS, SLOTS + 4):
            emit_pv(g - 4)
        emit_stage_copies(3, split=True)

        # ---- phase 5: proj (+bias via rank-1) + fused gated residual ----
        # partials (j=0..2) overlap the last pair normalize; j=3 + bias
        # lands once a_sb[3] is ready.
        proj_ps = {}
        gate64 = P.tile([128, 4], f32, tag="gate64")
        nc.vector.tensor_scalar_mul(gate64, mod_sb[:, 8:12], 1.0 / WSCALE)

        def pw3(jj, lo, hi):
            return pw8p[jj][:].rearrange("p (two f) -> p two f", two=2)[:, :, lo:hi]

        def a3(jj, lo, hi):
            return a8p[jj][:].rearrange("p (two f) -> p two f", two=2)[:, :, lo:hi]

        def proj_partial(m):
            # jj=0 (pairs 0,1) as DoubleRow + pair 2 as plain fp8: everything
            # that doesn't need the last pair's normalize
            ps = PSM.tile([128, T], f32, tag="sc", name=f"projps{m}")
            proj_ps[m] = ps
            for t in range(NT):
                nc.tensor.matmul(
                    ps[:, 512 * t : 512 * (t + 1)],
                    pw3(0, 128 * m, 128 * (m + 1)),
                    a3(0, 512 * t, 512 * (t + 1)),
                    start=True,
                    stop=False,
                    perf_mode=DR,
                )
            for t in range(NT):
                nc.tensor.matmul(
                    ps[:, 512 * t : 512 * (t + 1)],
                    pw8p[1][:, 128 * m : 128 * (m + 1)],
                    a8p[1][:, 512 * t : 512 * (t + 1)],
                    start=False,
                    stop=False,
                )

        def proj_finish(m):
            ps = proj_ps.pop(m)
            for t in range(NT):
                nc.tensor.matmul(
                    ps[:, 512 * t : 512 * (t + 1)],
                    pw8p[1][:, 512 + 128 * m : 512 + 128 * (m + 1)],
                    a8p[1][:, T + 512 * t : T + 512 * (t + 1)],
                    start=False,
                    stop=False,
                )
            for t in range(NT):
                nc.tensor.matmul(
                    ps[:, 512 * t : 512 * (t + 1)],
                    pbr_sb[0:1, 128 * m : 128 * (m + 1)],
                    ones_row[0:1, 512 * t : 512 * (t + 1)],
                    start=False,
                    stop=True,
                )
            # xf = xf + (gate/WSCALE) * ps   (ps = WSCALE*(proj + pb))
            nc.vector.scalar_tensor_tensor(
                xf[m][:], ps, gate64[:, m : m + 1], xf[m][:],
                ALU.mult, ALU.add,
            )
            nc.sync.dma_start(out=out_d.ap()[128 * m : 128 * (m + 1), :], in_=xf[m])

        emit_bc_recip(3)
        proj_partial(0)
        proj_partial(1)
        emit_muls(3)
        proj_finish(0)
        proj_partial(2)
        proj_finish(1)
        proj_partial(3)
        proj_finish(2)
        proj_finish(3)

    nc.compile()
    return nc


_PROGRAM = None
LAST_RESULTS = None


def _get_program():
    global _PROGRAM
    if _PROGRAM is None:
        _PROGRAM = _build_program()
    return _PROGRAM


def kernel(x, emb, qkv_w, qkv_b, ada_w, ada_b, proj_w, proj_b, _trace=False):
    global LAST_RESULTS
    import ml_dtypes

    nc = _get_program()

    x = np.asarray(x, np.float32)
    emb = np.asarray(emb, np.float32)
    perm = _perm()
    bf = ml_dtypes.bfloat16
    f8n = ml_dtypes.float8_e4m3

    def to_f8(a):
        return np.ascontiguousarray(np.clip(a * 64.0, -240.0, 240.0).astype(f8n))

    qkv_wT = to_f8(np.asarray(qkv_w, np.float32)[perm, :].T)
    qkv_b_p = np.ascontiguousarray(np.asarray(qkv_b, np.float32)[perm])
    vbrow = np.ascontiguousarray(qkv_b_p[1024:].astype(bf).reshape(1, C))
    ada_wT = to_f8(np.asarray(ada_w, np.float32).T)
    ada_b = np.ascontiguousarray(np.asarray(ada_b, np.float32))
    proj_wT = to_f8(np.asarray(proj_w, np.float32).T)
    pbrow = to_f8(np.asarray(proj_b, np.float32).reshape(1, C))

    gind = np.repeat(np.eye(8, dtype=np.float32), 16, axis=0) / 16.0  # [128, 8]
    gindT = np.ascontiguousarray(np.repeat(np.eye(8, dtype=np.float32), 16, axis=0).T)

    in_maps = []
    for b in range(B):
        in_maps.append(
            {
                "x": np.ascontiguousarray(x[b].reshape(C, T)),
                "emb": np.ascontiguousarray(emb[b]),
                "qkv_wT": qkv_wT,
                "qkv_b": qkv_b_p,
                "vbrow": vbrow,
                "ada_wT": ada_wT,
                "ada_b": ada_b,
                "proj_wT": proj_wT,
                "pbrow": pbrow,
                "gind": gind,
                "gindT": gindT,
            }
        )

    res = run_bass_kernel_spmd(nc, in_maps, list(range(8)), trace=_trace)
    LAST_RESULTS = res
    out = np.stack([res.results[b]["out"] for b in range(B)], axis=0)
    return np.ascontiguousarray(out.reshape(B, C, HH, WW).astype(np.float32))
